# revision 7
# baseline (speedup 1.0000x reference)
"""Trainium2 Bass kernel: CrossModalAttention (B=8, N=1024, E=768, H=8, D=96).

Sharding: data-parallel over batch B across 8 NeuronCores (1 batch element
per core). Weights replicated. No collectives.

Per-core dataflow (all matmuls bf16 with fp32 PSUM accumulation):
  A. Load fp32 inputs, cast to bf16, stage to DRAM scratch, xbar-DMA-transpose
     back to get X^T [e_in, m] and W^T [e_in, e_out] layouts in SBUF.
  B. Q/K/V projections (natural [token, e_out] layout); Q/K round-trip through
     DRAM scratch with per-head overlapping 128-col transposes to get
     QT_h/KT_h [d, m] (head-aligned, rows 0:96 valid). Biases for Q/K added
     post-transpose as per-partition scalars; V bias folded in later via
     sum(attn) == 1.
  C. Per head: S^T = K_h^T.T @ Q_h^T -> exp (ACT, unnormalized, bf16),
     PV matmul with a ones-column appended to V gives attended^T (rows 0:96)
     and row-sums (row 96) in one accumulation. Reciprocal of sums is
     DMA-broadcast across partitions; attn_avg^T accumulated in bf16.
  D. Output projection contracting per-head attT tiles against head-aligned
     o_w^T tiles; o_b added via a ones-row matmul.
  E. attn_avg^T staged to DRAM (bf16), xbar-transposed back to [m, p],
     scaled by 1/H, cast to fp32, stored.
"""

import sys

for _p in ("/opt/trn_rl_repo",):
    if _p not in sys.path:
        sys.path.insert(0, _p)

import numpy as np

import concourse.bass as bass
import concourse.mybir as mybir
import concourse.tile as tile
from concourse import bacc
from concourse.bass_utils import run_bass_kernel_spmd

F32 = mybir.dt.float32
BF16 = mybir.dt.bfloat16
AF = mybir.ActivationFunctionType
OP = mybir.AluOpType

B = 8
N = 1024  # tokens (both modalities)
E = 768
H = 8
D = 96
EPAD = 896  # padded width so per-head 128-col transpose slices stay in bounds
NCHUNK = N // 128  # 8
ECHUNK = E // 128  # 6
SCALE = 1.0 / float(np.sqrt(D))


def _body(nc, tc):
    xm = nc.dram_tensor("mammo_tokens", [N, E], F32, kind="ExternalInput").ap()
    xp = nc.dram_tensor("patho_tokens", [N, E], F32, kind="ExternalInput").ap()
    w_in = {}
    b_in = {}
    for nm in ("q", "k", "v", "o"):
        w_in[nm] = nc.dram_tensor(f"{nm}_w", [E, E], F32, kind="ExternalInput").ap()
        b_in[nm] = nc.dram_tensor(f"{nm}_b", [E], F32, kind="ExternalInput").ap()
    out_att = nc.dram_tensor("attended", [N, E], F32, kind="ExternalOutput").ap()
    out_avg = nc.dram_tensor("attn_avg", [N, N], F32, kind="ExternalOutput").ap()

    # DRAM scratch
    xm_bf = nc.dram_tensor("xm_bf", [N, E], BF16, kind="Internal").ap()
    xp_bf = nc.dram_tensor("xp_bf", [N, E], BF16, kind="Internal").ap()
    wbf = {
        nm: nc.dram_tensor(f"w{nm}_bf", [E, E], BF16, kind="Internal").ap()
        for nm in ("q", "k", "v")
    }
    wo_bf = nc.dram_tensor("wo_bf", [E, EPAD], BF16, kind="Internal").ap()
    q_scr = nc.dram_tensor("q_scr", [N, EPAD], BF16, kind="Internal").ap()
    k_scr = nc.dram_tensor("k_scr", [N, EPAD], BF16, kind="Internal").ap()
    avgT_d = nc.dram_tensor("avgT_scr", [N, N], BF16, kind="Internal").ap()

    with tc.tile_pool(name="persist", bufs=1) as persist:
        # ---------------- Phase A: load + cast + stage + transpose ----------
        with tc.tile_pool(name="prep", bufs=3) as prep:
            jobs = [
                (xm, xm_bf, NCHUNK),
                (xp, xp_bf, NCHUNK),
                (w_in["q"], wbf["q"], ECHUNK),
                (w_in["k"], wbf["k"], ECHUNK),
                (w_in["v"], wbf["v"], ECHUNK),
            ]
            for src, dst, nch in jobs:
                for r in range(nch):
                    t32 = prep.tile([128, E], F32, tag="ld32")
                    nc.sync.dma_start(t32[:], src[r * 128 : (r + 1) * 128, :])
                    t16 = prep.tile([128, E], BF16, tag="ld16")
                    nc.any.tensor_copy(t16[:], t32[:])
                    nc.sync.dma_start(dst[r * 128 : (r + 1) * 128, :], t16[:])
            for r in range(ECHUNK):
                t32 = prep.tile([128, E], F32, tag="ld32")
                nc.sync.dma_start(t32[:], w_in["o"][r * 128 : (r + 1) * 128, :])
                t16 = prep.tile([128, E], BF16, tag="ld16")
                nc.any.tensor_copy(t16[:], t32[:])
                nc.sync.dma_start(wo_bf[r * 128 : (r + 1) * 128, 0:E], t16[:])

            # biases: per-head [D, H] layout (bf16 via SWDGE cast-dma)
            qb_sb = persist.tile([D, H], F32, tag="qb")
            nc.sync.dma_start(qb_sb[:], b_in["q"].rearrange("(h d) -> d h", h=H))
            kb_sb = persist.tile([D, H], F32, tag="kb")
            nc.sync.dma_start(kb_sb[:], b_in["k"].rearrange("(h d) -> d h", h=H))
            vb_sb = persist.tile([D, H], F32, tag="vb")
            nc.sync.dma_start(vb_sb[:], b_in["v"].rearrange("(h d) -> d h", h=H))
            ob32 = prep.tile([1, E], F32, tag="ob32")
            nc.sync.dma_start(ob32[:], b_in["o"].rearrange("(a e) -> a e", a=1))
            ob_row = persist.tile([1, E], BF16, tag="ob")
            nc.any.tensor_copy(ob_row[:], ob32[:])
            ones_row = persist.tile([1, 128], BF16, tag="ones")
            nc.vector.memset(ones_row[:], 1.0)
            zpad = persist.tile([128, EPAD - E], BF16, tag="zpad")
            nc.vector.memset(zpad[:], 0.0)
            for r in range(ECHUNK):
                nc.sync.dma_start(wo_bf[r * 128 : (r + 1) * 128, E:EPAD], zpad[:])

        # transposed layouts (xbar dma transpose, bf16)
        xtwt_cm = tc.tile_pool(name="xtwt", bufs=1)
        xtwt = xtwt_cm.__enter__()
        XT_m = []
        XT_p = []
        for j in range(ECHUNK):
            t = xtwt.tile([128, N], BF16, tag=f"xtm{j}")
            nc.sync.dma_start_transpose(t[:], xm_bf[:, j * 128 : (j + 1) * 128])
            XT_m.append(t)
            t = xtwt.tile([128, N], BF16, tag=f"xtp{j}")
            nc.sync.dma_start_transpose(t[:], xp_bf[:, j * 128 : (j + 1) * 128])
            XT_p.append(t)
        WT = {}
        for nm in ("q", "k", "v"):
            WT[nm] = []
            for j in range(ECHUNK):
                t = xtwt.tile([128, E], BF16, tag=f"wt{nm}{j}")
                nc.sync.dma_start_transpose(t[:], wbf[nm][:, j * 128 : (j + 1) * 128])
                WT[nm].append(t)
        o_wT = []
        for h in range(H):
            t = persist.tile([128, E], BF16, tag=f"owt{h}")
            nc.sync.dma_start_transpose(t[:], wo_bf[:, h * D : h * D + 128])
            o_wT.append(t)

        # ---------------- Phase B: projections ------------------------------
        QT = [persist.tile([128, N], BF16, tag=f"qt{h}", name=f"qt{h}") for h in range(H)]
        KT = [persist.tile([128, N], BF16, tag=f"kt{h}", name=f"kt{h}") for h in range(H)]
        V_sb = [persist.tile([128, H, 128], BF16, tag=f"vsb{c}", name=f"vsb{c}") for c in range(NCHUNK)]

        with (
            tc.tile_pool(name="proj_ps", bufs=3, space=bass.MemorySpace.PSUM) as pps,
            tc.tile_pool(name="proj_sb", bufs=3) as psb,
        ):
            # Q (from mammo) and K (from patho): natural layout -> DRAM scratch
            for xt, wname, dst in ((XT_m, "q", q_scr), (XT_p, "k", k_scr)):
                for c in range(NCHUNK):
                    ps = pps.tile([128, E], F32, tag="pp")
                    for j in range(ECHUNK):
                        for lo, szz in ((0, 512), (512, 256)):
                            nc.tensor.matmul(
                                ps[:, lo : lo + szz],
                                xt[j][:, c * 128 : (c + 1) * 128],
                                WT[wname][j][:, lo : lo + szz],
                                start=(j == 0),
                                stop=(j == ECHUNK - 1),
                            )
                    t16 = psb.tile([128, E], BF16, tag="pb")
                    nc.any.tensor_copy(t16[:], ps[:])
                    nc.sync.dma_start(dst[c * 128 : (c + 1) * 128, 0:E], t16[:])
                    nc.sync.dma_start(dst[c * 128 : (c + 1) * 128, E:EPAD], zpad[:])
            # V: natural layout kept in SBUF, head-padded with ones column
            for c in range(NCHUNK):
                ps = pps.tile([128, E], F32, tag="pp")
                for j in range(ECHUNK):
                    for lo, szz in ((0, 512), (512, 256)):
                        nc.tensor.matmul(
                            ps[:, lo : lo + szz],
                            XT_p[j][:, c * 128 : (c + 1) * 128],
                            WT["v"][j][:, lo : lo + szz],
                            start=(j == 0),
                            stop=(j == ECHUNK - 1),
                        )
                nc.vector.memset(V_sb[c][:, :, D : D + 1], 1.0)
                nc.vector.memset(V_sb[c][:, :, D + 1 : 128], 0.0)
                nc.any.tensor_copy(
                    V_sb[c][:, :, 0:D], ps.rearrange("p (h d) -> p h d", h=H)
                )

        xtwt_cm.__exit__(None, None, None)

        # per-head transposed Q/K (overlapping 128-col slices; rows 0:96 valid)
        for h in range(H):
            nc.sync.dma_start_transpose(QT[h][:], q_scr[:, h * D : h * D + 128])
            nc.vector.tensor_scalar_add(QT[h][0:D, :], QT[h][0:D, :], qb_sb[:, h : h + 1])
            nc.sync.dma_start_transpose(KT[h][:], k_scr[:, h * D : h * D + 128])
            nc.vector.tensor_scalar_add(KT[h][0:D, :], KT[h][0:D, :], kb_sb[:, h : h + 1])

        # ---------------- Phase C: attention --------------------------------
        avgT = [persist.tile([128, N], BF16, tag=f"avg{c}", name=f"avg{c}") for c in range(NCHUNK)]
        attT = [persist.tile([D, N], BF16, tag=f"attT{h}", name=f"attT{h}") for h in range(H)]

        with (
            tc.tile_pool(name="sps", bufs=2, space=bass.MemorySpace.PSUM) as sps,
            tc.tile_pool(name="aps", bufs=2, space=bass.MemorySpace.PSUM) as aps,
            tc.tile_pool(name="expp", bufs=2) as expp,
            tc.tile_pool(name="cmisc", bufs=2) as cmisc,
        ):
            for h in range(H):
                attu = aps.tile([128, N], F32, tag="attu")
                exps = []
                for c in range(NCHUNK):
                    sp = sps.tile([128, N], F32, tag="spm")
                    for lo in (0, 512):
                        nc.tensor.matmul(
                            sp[:, lo : lo + 512],
                            KT[h][0:D, c * 128 : (c + 1) * 128],
                            QT[h][0:D, lo : lo + 512],
                            start=True,
                            stop=True,
                        )
                    ex = expp.tile([128, N], BF16, tag=f"exp{c}")
                    nc.scalar.activation(ex[:], sp[:], AF.Exp, scale=SCALE)
                    exps.append(ex)
                    for lo in (0, 512):
                        nc.tensor.matmul(
                            attu[:, lo : lo + 512],
                            V_sb[c][:, h, :],
                            ex[:, lo : lo + 512],
                            start=(c == 0),
                            stop=(c == NCHUNK - 1),
                        )
                # reciprocal of row-sums (landed in partition D of attu)
                rf = cmisc.tile([D + 1, N], F32, tag="rf")
                nc.vector.reciprocal(rf[D : D + 1, :], attu[D : D + 1, :])
                rf0 = cmisc.tile([1, N], F32, tag="rf0")
                nc.sync.dma_start(rf0[:], rf[D : D + 1, :])
                rb0 = cmisc.tile([1, N], BF16, tag="rb0")
                nc.vector.tensor_copy(rb0[:], rf0[:])
                recipBf = cmisc.tile([D, N], F32, tag="rBf")
                nc.gpsimd.partition_broadcast(recipBf[:], rf0[0:1, :])
                recipBb = cmisc.tile([128, N], BF16, tag="rBb")
                nc.gpsimd.partition_broadcast(recipBb[:], rb0[0:1, :])
                # attended^T: normalize + V-bias (sum(attn)==1)
                tmpn = cmisc.tile([D, N], BF16, tag="tmpn")
                nc.vector.tensor_tensor(tmpn[:], attu[0:D, :], recipBf[:], op=OP.mult)
                nc.vector.tensor_scalar_add(attT[h][:], tmpn[:], vb_sb[:, h : h + 1])
                # attn_avg^T accumulation
                for c in range(NCHUNK):
                    if h == 0:
                        nc.vector.tensor_tensor(
                            avgT[c][:], exps[c][:], recipBb[:], op=OP.mult
                        )
                    else:
                        t2 = cmisc.tile([128, N], BF16, tag=f"avt{c % 2}")
                        nc.vector.tensor_tensor(t2[:], exps[c][:], recipBb[:], op=OP.mult)
                        nc.gpsimd.tensor_tensor(avgT[c][:], avgT[c][:], t2[:], op=OP.add)

        # ---------------- Phase D: output projection ------------------------
        with (
            tc.tile_pool(name="ops", bufs=2, space=bass.MemorySpace.PSUM) as ops,
            tc.tile_pool(name="osb", bufs=3) as osb,
        ):
            for mc in range(NCHUNK):
                po = ops.tile([128, E], F32, tag="po")
                for h in range(H):
                    for lo, szz in ((0, 512), (512, 256)):
                        nc.tensor.matmul(
                            po[:, lo : lo + szz],
                            attT[h][:, mc * 128 : (mc + 1) * 128],
                            o_wT[h][0:D, lo : lo + szz],
                            start=(h == 0),
                            stop=False,
                        )
                for lo, szz in ((0, 512), (512, 256)):
                    nc.tensor.matmul(
                        po[:, lo : lo + szz],
                        ones_row[:, 0:128],
                        ob_row[:, lo : lo + szz],
                        start=False,
                        stop=True,
                    )
                os_ = osb.tile([128, E], F32, tag="os")
                nc.any.tensor_copy(os_[:], po[:])
                nc.sync.dma_start(out_att[mc * 128 : (mc + 1) * 128, :], os_[:])

            # ---------------- Phase E: attn_avg output ----------------------
            for c in range(NCHUNK):
                nc.sync.dma_start(avgT_d[c * 128 : (c + 1) * 128, :], avgT[c][:])
            for mc in range(NCHUNK):
                a16 = osb.tile([128, N], BF16, tag="a16")
                nc.sync.dma_start_transpose(a16[:], avgT_d[:, mc * 128 : (mc + 1) * 128])
                a32 = osb.tile([128, N], F32, tag="a32")
                nc.vector.tensor_scalar_mul(a32[:], a16[:], 1.0 / H)
                nc.sync.dma_start(out_avg[mc * 128 : (mc + 1) * 128, :], a32[:])


_NC_CACHE = {}


def build_nc():
    if "nc" not in _NC_CACHE:
        nc = bacc.Bacc("TRN2", target_bir_lowering=False, debug=False)
        with tile.TileContext(nc) as tc:
            _body(nc, tc)
        nc.compile()
        _NC_CACHE["nc"] = nc
    return _NC_CACHE["nc"]


def kernel(
    mammo_tokens,
    patho_tokens,
    q_w,
    q_b,
    k_w,
    k_b,
    v_w,
    v_b,
    o_w,
    o_b,
    _trace=False,
    _trace_kwargs=None,
):
    nc = build_nc()
    shared = {
        "q_w": np.ascontiguousarray(np.asarray(q_w, dtype=np.float32)),
        "q_b": np.ascontiguousarray(np.asarray(q_b, dtype=np.float32)),
        "k_w": np.ascontiguousarray(np.asarray(k_w, dtype=np.float32)),
        "k_b": np.ascontiguousarray(np.asarray(k_b, dtype=np.float32)),
        "v_w": np.ascontiguousarray(np.asarray(v_w, dtype=np.float32)),
        "v_b": np.ascontiguousarray(np.asarray(v_b, dtype=np.float32)),
        "o_w": np.ascontiguousarray(np.asarray(o_w, dtype=np.float32)),
        "o_b": np.ascontiguousarray(np.asarray(o_b, dtype=np.float32)),
    }
    mam = np.asarray(mammo_tokens, dtype=np.float32)
    pat = np.asarray(patho_tokens, dtype=np.float32)
    in_maps = []
    for b in range(B):
        m = dict(shared)
        m["mammo_tokens"] = np.ascontiguousarray(mam[b])
        m["patho_tokens"] = np.ascontiguousarray(pat[b])
        in_maps.append(m)
    kw = dict(_trace_kwargs or {})
    res = run_bass_kernel_spmd(
        nc, in_maps, core_ids=list(range(B)), trace=_trace, **kw
    )
    attended = np.stack([res.results[b]["attended"] for b in range(B)])
    attn_avg = np.stack([res.results[b]["attn_avg"] for b in range(B)])
    if _trace:
        return (attended, attn_avg), res
    return attended, attn_avg
